# revision 10
# baseline (speedup 1.0000x reference)
"""Trainium2 Bass kernel for the moe_routing classifier problem.

Computation (per batch row b, class c):
  cos[b,c,s]  = cosine(emb[b], weight[c,s])            (64 sub-prototypes)
  top-8 over s, softmax weights w, protos = sum_k w_k * weight[c, idx_k]
  out[b,c]    = ((1 + cosine(protos, emb[b])) / 2 + 1e-8) / 0.1

Key algebra used by the kernel (avoids gathers entirely):
  E[b,c,s]   = exp(score) masked to the top-8 entries (unnormalized softmax)
  dot2*Z     = sum_s E * dot_raw                        (Z cancels later)
  |protos|^2*Z^2 = E^T (W W^T) E  via per-class Gram matrices
  cos2       = (sum_s E*dot_raw) * inv|emb| / sqrt(E^T G E)

Sharding: classes are split across the 8 cores (32 classes each); emb is
replicated. Each core writes a [1024, 32] slice of the output.
"""

import numpy as np

B, D, C, S = 1024, 128, 256, 64
NCORES = 8
C_LOC = C // NCORES        # 32 classes per core
CS = C_LOC * S             # 2048 anchor rows per core
P = 128                    # partitions
NBT = B // P               # 8 batch tiles
NWT = CS // P              # 16 weight tiles
EPS = 1e-8
SC_BIAS = 0.5 + EPS        # score = 0.5*cos + SC_BIAS
OUT_SCALE = 5.0            # ((1+x)/2 + 1e-8) / 0.1 = 5x + 5 + 1e-7
OUT_BIAS = 5.0 + 1e-7

PAIR_TRANSPOSE = True      # transpose two 64-wide classes per PE op

_CACHE = {}


def build_nc(stage=99.0):
    import concourse.bass as bass
    import concourse.tile as tile
    from concourse import bacc, mybir
    from concourse.masks import make_identity
    from contextlib import ExitStack

    f32 = mybir.dt.float32
    AF = mybir.ActivationFunctionType

    nc = bacc.Bacc(None, target_bir_lowering=False)
    emb_d = nc.dram_tensor("emb", [B, D], f32, kind="ExternalInput")
    w_d = nc.dram_tensor("weight", [CS, D], f32, kind="ExternalInput")
    out_d = nc.dram_tensor("out", [B, C_LOC], f32, kind="ExternalOutput")

    with tile.TileContext(nc) as tc, ExitStack() as ctx:
        sing = ctx.enter_context(tc.tile_pool(name="sing", bufs=1))
        dram = ctx.enter_context(tc.tile_pool(name="dram", bufs=1, space="DRAM"))
        work = ctx.enter_context(tc.tile_pool(name="work", bufs=2))
        small = ctx.enter_context(tc.tile_pool(name="small", bufs=4))
        jk = ctx.enter_context(tc.tile_pool(name="jk", bufs=8))
        fpool = ctx.enter_context(tc.tile_pool(name="fpool", bufs=4))
        opool = ctx.enter_context(tc.tile_pool(name="opool", bufs=3))
        ps_mm = ctx.enter_context(tc.tile_pool(name="ps_mm", bufs=1, space="PSUM"))
        ps_tr = ctx.enter_context(tc.tile_pool(name="ps_tr", bufs=2, space="PSUM"))
        ps_eg = ctx.enter_context(tc.tile_pool(name="ps_eg", bufs=2, space="PSUM"))

        ident = sing.tile([P, P], f32)
        make_identity(nc, ident[:])
        sbias = sing.tile([P, 1], f32)     # score bias as AP (Exp needs AP bias)
        nc.vector.memset(sbias[:], SC_BIAS)

        # ---------------- load inputs ----------------
        Wn = sing.tile([P, NWT, D], f32)   # weight rows, tiled by 128
        nc.sync.dma_start(Wn[:], w_d[:].rearrange("(t p) d -> p t d", p=P))
        En = sing.tile([P, NBT, D], f32)   # emb rows, tiled by 128
        nc.sync.dma_start(En[:], emb_d[:].rearrange("(t p) d -> p t d", p=P))

        # ---------------- norms ----------------
        nwsq = sing.tile([P, NWT], f32)    # ||w_row||^2, row-tiled layout
        for t in range(NWT):
            j = jk.tile([P, D], f32, tag="jact")
            nc.scalar.activation(j[:], Wn[:, t], AF.Square,
                                 accum_out=nwsq[:, t : t + 1])
        esq = sing.tile([P, NBT], f32)
        for t in range(NBT):
            j = jk.tile([P, D], f32, tag="jact")
            nc.scalar.activation(j[:], En[:, t], AF.Square,
                                 accum_out=esq[:, t : t + 1])
        ne = sing.tile([P, NBT], f32)      # ||emb||
        nc.scalar.activation(ne[:], esq[:], AF.Sqrt)
        ine = sing.tile([P, NBT], f32)     # 1/||emb||
        nc.vector.reciprocal(ine[:], ne[:])
        hine = sing.tile([P, NBT], f32)    # 0.5/||emb||
        nc.vector.tensor_scalar_mul(hine[:], ine[:], 0.5)

        if stage < 2:
            for bt in range(NBT):
                osb = opool.tile([P, C_LOC], f32, tag="osb")
                nc.vector.memset(osb[:], 1.0)
                nc.vector.tensor_scalar_mul(osb[:, :NBT], ine[:], 1.0)
                nc.sync.dma_start(out_d[bt * P : (bt + 1) * P, :], osb[:])
            nc.compile()
            return nc

        # nw broadcast rows: roundtrip through DRAM to reorder + partition-bcast
        scr = dram.tile([CS], f32)
        nc.sync.dma_start(scr[:].rearrange("(t p) -> p t", p=P), nwsq[:])
        scr_bc = bass.AP(
            tensor=scr[:].tensor, offset=scr[:].offset,
            ap=[[0, P]] + list(scr[:].ap),
        )
        NWB = sing.tile([P, CS], f32)      # ||w_row|| broadcast over partitions
        INWB = sing.tile([P, CS], f32)     # 1/||w_row||
        nc.sync.dma_start(NWB[:], scr_bc)
        nc.scalar.activation(NWB[:], NWB[:], AF.Sqrt)
        nc.vector.reciprocal(INWB[:], NWB[:])

        if stage < 3:
            for bt in range(NBT):
                osb = opool.tile([P, C_LOC], f32, tag="osb")
                nc.vector.tensor_copy(osb[:], NWB[:, :C_LOC])
                nc.vector.tensor_mul(osb[:], osb[:], INWB[:, :C_LOC])
                nc.sync.dma_start(out_d[bt * P : (bt + 1) * P, :], osb[:])
            nc.compile()
            return nc

        # ---------------- transposed operands ----------------
        WT = sing.tile([P, CS], f32)       # W^T  [d, cs]
        for t in range(NWT):
            pst = ps_tr.tile([P, P], f32, tag="tr")
            nc.tensor.transpose(pst[:], Wn[:, t], ident[:])
            nc.scalar.copy(WT[:, t * P : (t + 1) * P], pst[:])
        VT = sing.tile([P, CS], f32)       # normalized anchors transposed
        nc.vector.tensor_mul(VT[:], WT[:], INWB[:])

        embT = sing.tile([P, B], f32)      # emb^T [d, b]
        for t in range(NBT):
            pst = ps_tr.tile([P, P], f32, tag="tr")
            nc.tensor.transpose(pst[:], En[:, t], ident[:])
            nc.scalar.copy(embT[:, t * P : (t + 1) * P], pst[:])

        # per-class raw Gram matrices G_c = W_c W_c^T  [64, 64], then packed
        # into block-diagonal pair matrices GP[:, q*128:(q+1)*128] =
        # [[G_2q, 0], [0, G_2q+1]] so one full-size (0,0)-quadrant matmul
        # computes EG for a transposed class pair (quadrant matmuls
        # interleaved with transposes crash the device).
        Gtmp = sing.tile([S, CS], f32)
        for c in range(C_LOC):
            cs = slice(c * S, (c + 1) * S)
            psg = ps_tr.tile([P, P], f32, tag="tr")
            nc.tensor.matmul(psg[:S, :S], WT[:, cs], WT[:, cs])
            nc.scalar.copy(Gtmp[:, cs], psg[:S, :S])
        GP = sing.tile([P, CS], f32)
        nc.vector.memset(GP[:], 0.0)
        gt3 = Gtmp[:].rearrange("p (q j) -> p q j", j=2 * S)
        gp3 = GP[:].rearrange("p (q j) -> p q j", j=2 * S)
        nc.sync.dma_start(gp3[0:S, :, 0:S], gt3[:, :, 0:S])
        nc.sync.dma_start(gp3[S : 2 * S, :, S : 2 * S], gt3[:, :, S : 2 * S])

        if stage < 4:
            for bt in range(NBT):
                osb = opool.tile([P, C_LOC], f32, tag="osb")
                nc.vector.tensor_copy(osb[:], VT[:, :C_LOC])
                nc.vector.tensor_mul(osb[:], osb[:], G[:, :C_LOC].to_broadcast([P, C_LOC]) if False else osb[:])
                nc.sync.dma_start(out_d[bt * P : (bt + 1) * P, :], osb[:])
            nc.compile()
            return nc

        # ---------------- main loop over batch tiles ----------------
        for bt in range(NBT):
            bsl = slice(bt * P, (bt + 1) * P)
            dotn = ps_mm.tile([P, CS], f32, tag="mm")
            for j in range(CS // 512):
                nc.tensor.matmul(
                    dotn[:, j * 512 : (j + 1) * 512],
                    embT[:, bsl],
                    VT[:, j * 512 : (j + 1) * 512],
                )
            # exp of score, fused with cos normalization + affine
            exps = work.tile([P, CS], f32, tag="exps")
            for j in range(CS // 512):
                nc.scalar.activation(
                    exps[:, j * 512 : (j + 1) * 512],
                    dotn[:, j * 512 : (j + 1) * 512],
                    AF.Exp,
                    bias=sbias[:],
                    scale=hine[:, bt : bt + 1],
                )
            if stage < 5:
                osb = opool.tile([P, C_LOC], f32, tag="osb")
                nc.vector.tensor_copy(osb[:], exps[:, :C_LOC])
                nc.sync.dma_start(out_d[bsl, :], osb[:])
                continue

            # raw dots (for dot2): dotn * ||w||
            dotr = work.tile([P, CS], f32, tag="dotr")
            nc.vector.tensor_mul(dotr[:], dotn[:], NWB[:])

            # top-8 selection per class: R = exps with top8 zeroed
            R = work.tile([P, CS], f32, tag="R")
            for c in range(C_LOC):
                cs = slice(c * S, (c + 1) * S)
                mx8 = small.tile([P, 8], f32, tag="mx8")
                nc.vector.max(out=mx8[:], in_=exps[:, cs])
                nc.vector.match_replace(
                    out=R[:, cs], in_to_replace=mx8[:],
                    in_values=exps[:, cs], imm_value=0.0,
                )
            E = work.tile([P, CS], f32, tag="E")
            nc.vector.tensor_sub(E[:], exps[:], R[:])

            if stage < 6:
                osb = opool.tile([P, C_LOC], f32, tag="osb")
                nc.vector.tensor_copy(osb[:], E[:, :C_LOC])
                nc.vector.tensor_mul(osb[:], osb[:], dotr[:, :C_LOC])
                nc.sync.dma_start(out_d[bsl, :], osb[:])
                continue

            # E^T per class (pairs of 64-wide classes per PE transpose),
            # EG = E_c @ G_c accumulated in 512-wide PSUM chunks (8 classes),
            # then one elementwise mul + segmented reduce per quantity.
            d2z = small.tile([P, C_LOC], f32, tag="d2z")
            np2z = small.tile([P, C_LOC], f32, tag="np2z")
            prod_d = work.tile([P, CS], f32, tag="prod_d")
            nc.vector.tensor_mul(prod_d[:], E[:], dotr[:])
            nc.vector.tensor_reduce(
                d2z[:], prod_d[:].rearrange("p (c s) -> p c s", c=C_LOC),
                axis=mybir.AxisListType.X, op=mybir.AluOpType.add)
            prod_n = work.tile([P, CS], f32, tag="prod_n")
            for q8 in range(CS // 512):
                pse = ps_eg.tile([P, 512], f32, tag="eg")
                for qq in range(4):
                    q = 4 * q8 + qq
                    qs = slice(q * 128, (q + 1) * 128)
                    pst = ps_tr.tile([P, P], f32, tag="tr")
                    nc.tensor.transpose(pst[:], E[:, qs], ident[:])
                    F = fpool.tile([P, P], f32, tag="F")
                    nc.scalar.copy(F[:], pst[:])
                    nc.tensor.matmul(
                        pse[:, qq * 128 : (qq + 1) * 128], F[:], GP[:, qs]
                    )
                nc.vector.tensor_mul(
                    prod_n[:, q8 * 512 : (q8 + 1) * 512],
                    pse[:], E[:, q8 * 512 : (q8 + 1) * 512],
                )
            nc.vector.tensor_reduce(
                np2z[:], prod_n[:].rearrange("p (c s) -> p c s", c=C_LOC),
                axis=mybir.AxisListType.X, op=mybir.AluOpType.add)

            # cos2 = d2z * ine / sqrt(np2z);  out = 5*cos2 + 5 + 1e-7
            rnp = small.tile([P, C_LOC], f32, tag="rnp")
            nc.scalar.activation(rnp[:], np2z[:], AF.Sqrt)
            nc.vector.reciprocal(rnp[:], rnp[:])
            c2 = small.tile([P, C_LOC], f32, tag="c2")
            nc.vector.tensor_mul(c2[:], d2z[:], rnp[:])
            nc.vector.tensor_scalar_mul(c2[:], c2[:], ine[:, bt : bt + 1])
            osb = opool.tile([P, C_LOC], f32, tag="osb")
            nc.scalar.activation(osb[:], c2[:], AF.Copy,
                                 bias=OUT_BIAS, scale=OUT_SCALE)
            nc.sync.dma_start(out_d[bsl, :], osb[:])

    nc.compile()
    return nc


def _get_nc():
    if "nc" not in _CACHE:
        _CACHE["nc"] = build_nc()
    return _CACHE["nc"]


def kernel(emb: np.ndarray, weight: np.ndarray) -> np.ndarray:
    from concourse.bass_utils import run_bass_kernel_spmd

    emb = np.ascontiguousarray(np.asarray(emb, dtype=np.float32))
    weight = np.ascontiguousarray(np.asarray(weight, dtype=np.float32))
    assert emb.shape == (B, D) and weight.shape == (C, S, D)

    nc = _get_nc()
    in_maps = [
        {
            "emb": emb,
            "weight": np.ascontiguousarray(
                weight[i * C_LOC : (i + 1) * C_LOC].reshape(CS, D)
            ),
        }
        for i in range(NCORES)
    ]
    res = run_bass_kernel_spmd(nc, in_maps, core_ids=list(range(NCORES)))
    return np.concatenate(
        [res.results[i]["out"] for i in range(NCORES)], axis=1
    )
